# revision 1
# baseline (speedup 1.0000x reference)
"""Trainium2 Bass kernel for nn_ExecPolicyNetwork (ragged gather + 36-64-64-32-1 MLP).

Strategy (data-parallel over ragged rows, per sharding hint):
  * Host: replicate the reference's index math exactly (jax clamp-gather +
    jnp.repeat total_repeat_length semantics) with numpy, build the dense
    [36, T] feature-major input matrix, shard T across 8 cores.
  * Device (per core): stream [36, 512] column tiles; 4 back-to-back matmuls
    (feature-major activations) with bias+relu fused into the PSUM->SBUF
    evacuation on the Scalar (ACT) and Vector (DVE) engines.
  * Host: concatenate per-core score vectors, strip padding.
"""

import numpy as np

NUM_EXECUTORS = 50
NUM_DAG_FEATURES = 3
N_CORES = 8
TILE_N = 512

_NC_CACHE = {}


# --------------------------------------------------------------------------
# host-side index math (mirrors jax semantics exactly; validated vs reference)
# --------------------------------------------------------------------------

def _build_inputs(x, h_dag, h_glob, ptr, job_indices, num_exec_acts, exec_act_idx):
    x = np.asarray(x, dtype=np.float32)
    h_dag = np.asarray(h_dag, dtype=np.float32)
    h_glob = np.asarray(h_glob, dtype=np.float32)
    ptr = np.asarray(ptr).astype(np.int64)
    job_indices = np.asarray(job_indices).astype(np.int64)
    num_exec_acts = np.asarray(num_exec_acts).astype(np.int64)
    exec_act_idx = np.asarray(exec_act_idx).astype(np.int64)

    J = job_indices.shape[0]
    T = exec_act_idx.shape[0]
    n_nodes = x.shape[0]
    B = h_dag.shape[0]

    ji = np.clip(job_indices, 0, B - 1)                      # jax gathers clamp
    start_nodes = np.clip(ptr[:-1], 0, n_nodes - 1)
    x_dag = x[start_nodes[ji], :NUM_DAG_FEATURES]            # [J, 3]
    h_dag_sel = h_dag[ji]                                    # [J, 16]
    n_sel = num_exec_acts[ji]                                # [J]

    # jnp.repeat(arange(J), n_sel, total_repeat_length=T) emulation
    exclusive = np.roll(n_sel, 1)
    exclusive[0] = 0
    scatter = np.cumsum(exclusive)
    ind = np.zeros(T, np.int64)
    np.add.at(ind, scatter[scatter < T], 1)                  # OOB scatters drop
    rpt = np.cumsum(ind) - 1
    np.clip(rpt, 0, J - 1, out=rpt)

    U = np.concatenate([x_dag, h_dag_sel, h_glob], axis=1)   # [J, 35] fp32
    exec_col = exec_act_idx.astype(np.float32) * (1.0 / NUM_EXECUTORS)
    return U, rpt, exec_col, T


# --------------------------------------------------------------------------
# device kernel
# --------------------------------------------------------------------------

def _build_nc(ntiles):
    import concourse.bacc as bacc
    import concourse.tile as tile
    from concourse import mybir

    f32 = mybir.dt.float32
    Relu = mybir.ActivationFunctionType.Relu
    Add = mybir.AluOpType.add
    Max = mybir.AluOpType.max
    tcpad = ntiles * TILE_N

    nc = bacc.Bacc("TRN2", target_bir_lowering=False, debug=False)
    x36 = nc.dram_tensor("x36", [36, tcpad], f32, kind="ExternalInput")
    w1 = nc.dram_tensor("w1", [36, 64], f32, kind="ExternalInput")
    w2 = nc.dram_tensor("w2", [64, 64], f32, kind="ExternalInput")
    w3 = nc.dram_tensor("w3", [64, 32], f32, kind="ExternalInput")
    w4 = nc.dram_tensor("w4", [32, 1], f32, kind="ExternalInput")
    b1 = nc.dram_tensor("b1", [64, 1], f32, kind="ExternalInput")
    b2 = nc.dram_tensor("b2", [64, 1], f32, kind="ExternalInput")
    b3 = nc.dram_tensor("b3", [32, 1], f32, kind="ExternalInput")
    b4 = nc.dram_tensor("b4", [1, 1], f32, kind="ExternalInput")
    out = nc.dram_tensor("out", [ntiles, TILE_N], f32, kind="ExternalOutput")

    with tile.TileContext(nc) as tc:
        with (
            tc.tile_pool(name="singles", bufs=1) as singles,
            tc.tile_pool(name="xin", bufs=4) as xin,
            tc.tile_pool(name="h", bufs=3) as hp,
            tc.tile_pool(name="ps", bufs=2, space="PSUM") as pp,
            tc.tile_pool(name="o", bufs=4) as op,
        ):
            w1s = singles.tile([36, 64], f32, tag="w1")
            w2s = singles.tile([64, 64], f32, tag="w2")
            w3s = singles.tile([64, 32], f32, tag="w3")
            w4s = singles.tile([32, 1], f32, tag="w4")
            b1s = singles.tile([64, 1], f32, tag="b1")
            b2s = singles.tile([64, 1], f32, tag="b2")
            b3s = singles.tile([32, 1], f32, tag="b3")
            b4s = singles.tile([1, 1], f32, tag="b4")
            for sb, dr in ((w1s, w1), (w2s, w2), (w3s, w3), (w4s, w4),
                           (b1s, b1), (b2s, b2), (b3s, b3), (b4s, b4)):
                nc.sync.dma_start(sb[:], dr.ap())

            for i in range(ntiles):
                xt = xin.tile([36, TILE_N], f32, tag="x")
                nc.sync.dma_start(xt[:], x36.ap()[:, i * TILE_N:(i + 1) * TILE_N])

                ps1 = pp.tile([64, TILE_N], f32, tag="ps1")
                nc.tensor.matmul(ps1[:], w1s[:], xt[:], start=True, stop=True)
                h1 = hp.tile([64, TILE_N], f32, tag="h1")
                nc.scalar.activation(h1[:], ps1[:], Relu, bias=b1s[:])

                ps2 = pp.tile([64, TILE_N], f32, tag="ps2")
                nc.tensor.matmul(ps2[:], w2s[:], h1[:], start=True, stop=True)
                h2 = hp.tile([64, TILE_N], f32, tag="h2")
                nc.vector.tensor_scalar(out=h2[:], in0=ps2[:], scalar1=b2s[:],
                                        scalar2=0.0, op0=Add, op1=Max)

                ps3 = pp.tile([32, TILE_N], f32, tag="ps3")
                nc.tensor.matmul(ps3[:], w3s[:], h2[:], start=True, stop=True)
                h3 = hp.tile([32, TILE_N], f32, tag="h3")
                nc.scalar.activation(h3[:], ps3[:], Relu, bias=b3s[:])

                ps4 = pp.tile([1, TILE_N], f32, tag="ps4")
                nc.tensor.matmul(ps4[:], w4s[:], h3[:], start=True, stop=True)
                ot = op.tile([1, TILE_N], f32, tag="o")
                nc.vector.tensor_scalar(out=ot[:], in0=ps4[:], scalar1=b4s[:],
                                        scalar2=None, op0=Add)
                nc.sync.dma_start(out.ap()[i:i + 1, :], ot[:])

    nc.compile()
    return nc


def _get_nc(ntiles):
    if ntiles not in _NC_CACHE:
        _NC_CACHE[ntiles] = _build_nc(ntiles)
    return _NC_CACHE[ntiles]


# --------------------------------------------------------------------------
# entry point
# --------------------------------------------------------------------------

def kernel(x, h_dag, h_glob, ptr, job_indices, num_exec_acts, exec_act_idx,
           W1, b1, W2, b2, W3, b3, W4, b4):
    from concourse.bass_utils import run_bass_kernel_spmd

    U, rpt, exec_col, T = _build_inputs(
        x, h_dag, h_glob, ptr, job_indices, num_exec_acts, exec_act_idx)

    ntiles = -(-T // (N_CORES * TILE_N))          # per-core tile count
    tcpad = ntiles * TILE_N
    tpad = N_CORES * tcpad

    # feature-major [36, tpad]: rows 0..34 = U columns gathered by rpt, row 35 = exec
    X = np.zeros((36, tpad), np.float32)
    UT = np.ascontiguousarray(U.T)                # [35, J]
    X[:35, :T] = UT[:, rpt]
    X[35, :T] = exec_col

    common = {
        "w1": np.ascontiguousarray(np.asarray(W1, np.float32)),
        "w2": np.ascontiguousarray(np.asarray(W2, np.float32)),
        "w3": np.ascontiguousarray(np.asarray(W3, np.float32)),
        "w4": np.ascontiguousarray(np.asarray(W4, np.float32)),
        "b1": np.asarray(b1, np.float32).reshape(64, 1),
        "b2": np.asarray(b2, np.float32).reshape(64, 1),
        "b3": np.asarray(b3, np.float32).reshape(32, 1),
        "b4": np.asarray(b4, np.float32).reshape(1, 1),
    }
    in_maps = []
    for c in range(N_CORES):
        m = dict(common)
        m["x36"] = np.ascontiguousarray(X[:, c * tcpad:(c + 1) * tcpad])
        in_maps.append(m)

    nc = _get_nc(ntiles)
    res = run_bass_kernel_spmd(nc, in_maps, core_ids=list(range(N_CORES)))
    scores = np.concatenate([r["out"].reshape(-1) for r in res.results])
    return scores[:T].astype(np.float32)
